# revision 13
# baseline (speedup 1.0000x reference)
"""GP posterior mean mu = K_rbf(X_test, X_train) @ alpha on 8 NeuronCores,
exploiting the locality of the RBF kernel (lengthscale 0.1 on N(0,1) data).

Math per pair-block: K[j,i] = sf2 * exp(-0.5*||xt_i - x_j||^2 / ell2), with the
exponent expressed as a single 14-term dot product built from bf16 hi/lo splits
of the fp32 operands (zero-padded to a 128 contraction so the PE streams at the
full 2.4 GHz clock).  ScalarE applies exp, and a second TensorE matmul
contracts K against hi/lo-split alpha, accumulating in PSUM.

Sparsity: test and train points are sorted into compact spatial tiles by
recursive median bisection (host side).  Only (test-chunk, train-tile) block
pairs whose bounding boxes come within exponent TAU of each other are computed
-- at lengthscale 0.1 that is ~15% of all blocks.  The SPMD program is a
uniform grid of S slots x T train-tiles per core; heavy test chunks are split
across slots (partial sums merged on host) and the remainder is padded with
zero-alpha tiles, so every core runs the identical instruction stream and only
the gathered tile DATA differs per core.
"""

import math

import numpy as np
import ml_dtypes

M = 16384
N = 16384
NCORES = 8
TC = 256                  # test points per chunk (columns per slot)
TT = 128                  # train points per tile (one PE contraction)
TAU = 6.0                 # drop blocks with min exponent magnitude > TAU
G = 4                     # train tiles covered by one ACT instruction
C = 14                    # used contraction rows of the exponent matmul
CD = 32                   # contraction rows carried by DMA (32-partition align)
CP = 128                  # padded contraction (keeps PE at full clock)

_cache = {}


def _split2(v):
    hi = v.astype(ml_dtypes.bfloat16)
    lo = (v - hi.astype(np.float64)).astype(ml_dtypes.bfloat16)
    return hi, lo


def _split3(v):
    hi = v.astype(ml_dtypes.bfloat16)
    r = v - hi.astype(np.float64)
    mid = r.astype(ml_dtypes.bfloat16)
    lo = (r - mid.astype(np.float64)).astype(ml_dtypes.bfloat16)
    return hi, mid, lo


def _kd_perm(X, leaf):
    """Permutation sorting rows of X into contiguous leaves of size `leaf`
    via recursive median bisection (balanced: len(X) must be leaf * 2^k)."""
    out = []

    def rec(idx):
        if len(idx) <= leaf:
            out.append(idx)
            return
        P = X[idx]
        ax = int(np.argmax(P.max(0) - P.min(0)))
        order = np.argsort(P[:, ax], kind="stable")
        h = len(idx) // 2
        rec(idx[order[:h]])
        rec(idx[order[h:]])

    rec(np.arange(len(X)))
    return np.concatenate(out)


def _schedule(Xs, Xr, ell2):
    """Block-sparse schedule. Returns (perm_t, perm_r, S, T, entries) where
    entries is a list of 8*S (leaf_idx, tile_list) pairs in (slot-major,
    core-minor) order; leaf_idx may repeat (split chunks) or be -1 (empty)."""
    perm_t = _kd_perm(Xs, TC)
    perm_r = _kd_perm(Xr, TT)
    Xs_s, Xr_s = Xs[perm_t], Xr[perm_r]
    nt, nr = M // TC, N // TT
    tb_lo = Xs_s.reshape(nt, TC, 2).min(1)
    tb_hi = Xs_s.reshape(nt, TC, 2).max(1)
    rb_lo = Xr_s.reshape(nr, TT, 2).min(1)
    rb_hi = Xr_s.reshape(nr, TT, 2).max(1)
    gap = np.maximum(0.0, np.maximum(tb_lo[:, None] - rb_hi[None, :],
                                     rb_lo[None, :] - tb_hi[:, None]))
    d2 = (gap ** 2).sum(-1)
    need = d2 < 2.0 * ell2 * TAU  # (nt, nr)
    tiles_of = [np.nonzero(need[j])[0] for j in range(nt)]
    total = int(sum(len(t) for t in tiles_of))
    total = max(total, 1)

    best = None
    for S in range(max(1, nt // NCORES), 8 * max(1, nt // NCORES) + 9):
        T = max(1, math.ceil(total / (NCORES * S)))
        while sum(max(1, math.ceil(len(t) / T)) for t in tiles_of) > NCORES * S:
            T += 1
        # ACT cycles incl. per-instruction overhead, with ragged last group
        ngroups = math.ceil(T / G)
        act = S * sum(
            (min(G, T - g * G)) * TC + 222 for g in range(ngroups)
        )
        cost = (max(act / 1.2, S * T * TC * 2 / 2.4), S * T, S)
        if best is None or cost < best[0]:
            best = (cost, S, T)
    _, S, T = best

    entries = []
    for j in range(nt):
        t = tiles_of[j]
        for a in range(0, max(len(t), 1), T):
            entries.append((j, t[a:a + T]))
    while len(entries) < NCORES * S:
        entries.append((-1, np.array([], dtype=np.int64)))
    return perm_t, perm_r, S, T, entries


def _build_program(bias, S, T):
    import concourse.mybir as mybir
    import concourse.tile as tile
    from concourse import bacc

    fp32 = mybir.dt.float32
    bf16 = mybir.dt.bfloat16
    P = S * T
    groups = [(g, min(g + G, T)) for g in range(0, T, G)]
    W = TC + T * TT + T * 4      # columns per combined per-slot input tile

    nc = bacc.Bacc(None, target_bir_lowering=False)
    IN_d = nc.declare_dram_parameter("inp", [CP, S * W], bf16, isOutput=False)
    OUT_d = nc.declare_dram_parameter("out", [4, S * TC], fp32, isOutput=True)

    with tile.TileContext(nc) as tc:
        with (
            tc.tile_pool(name="singles", bufs=1) as singles,
            tc.tile_pool(name="kpool", bufs=3) as kpool,
            tc.tile_pool(name="opool", bufs=2) as opool,
            tc.tile_pool(name="pse", bufs=3, space="PSUM") as pse,
            tc.tile_pool(name="psacc", bufs=2, space="PSUM") as psacc,
        ):
            ins = []
            for s in range(S):
                t_in = singles.tile([CP, W], bf16, name=f"in{s}")
                eng = nc.sync if s % 2 == 0 else nc.gpsimd
                eng.dma_start(out=t_in, in_=IN_d[:, s * W:(s + 1) * W])
                ins.append(t_in)

            for s in range(S):
                acc = psacc.tile([4, TC], fp32)
                rhsB = ins[s][:, :TC]
                sb_A = ins[s][:, TC:TC + T * TT]
                sb_AL = ins[s][:TT, TC + T * TT:]
                for g0, g1 in groups:
                    e = pse.tile([128, (g1 - g0) * TC], fp32)
                    for t in range(g0, g1):
                        nc.tensor.matmul(
                            e[:, (t - g0) * TC:(t - g0 + 1) * TC],
                            lhsT=sb_A[:, t * TT:(t + 1) * TT],
                            rhs=rhsB,
                            start=True,
                            stop=True,
                        )
                    k = kpool.tile([128, (g1 - g0) * TC], bf16)
                    nc.scalar.activation(
                        k, e, mybir.ActivationFunctionType.Exp, bias=float(bias)
                    )
                    for t in range(g0, g1):
                        nc.tensor.matmul(
                            acc,
                            lhsT=sb_AL[:, t * 4:(t + 1) * 4],
                            rhs=k[:, (t - g0) * TC:(t - g0 + 1) * TC],
                            start=(t == 0),
                            stop=(t == T - 1),
                        )
                o = opool.tile([4, TC], fp32, name=f"o{s}")
                nc.vector.tensor_copy(o, acc)
                eng = nc.gpsimd if s % 2 == 0 else nc.sync
                eng.dma_start(out=OUT_d[:, s * TC:(s + 1) * TC], in_=o)
    nc.compile()
    return nc


def _prep(X_test, X_train, alpha, log_lengthscale, log_outputscale):
    ell = np.exp(np.float32(log_lengthscale))
    ell2 = np.float64(np.float32(ell) ** 2)
    sf = np.exp(np.float32(log_outputscale))
    sf2 = np.float64(np.float32(sf) ** 2)
    bias = np.float32(np.log(sf2))

    perm_t, perm_r, S, T, entries = _schedule(
        np.asarray(X_test, np.float64), np.asarray(X_train, np.float64), ell2
    )
    P = S * T

    xt = X_train.astype(np.float64)[perm_r]
    xs = X_test.astype(np.float64)[perm_t]
    al = alpha.astype(np.float64)[perm_r]

    # Train-side matrix A (CP, N); rows 14.. are zero padding
    x0h, x0l = _split2(xt[:, 0])
    x1h, x1l = _split2(xt[:, 1])
    pj = -(xt[:, 0] ** 2 + xt[:, 1] ** 2) / (2.0 * ell2)
    pjh, pjm, pjl = _split3(pj)
    ones = np.ones(N, dtype=ml_dtypes.bfloat16)
    A = np.stack(
        [ones, ones, ones, x0h, x0h, x0l, x0l, x1h, x1h, x1l, x1l, pjh, pjm, pjl]
    )

    # Test-side matrix B (CP, M); rows 14.. are zero padding
    T0 = -(xs[:, 0] ** 2 + xs[:, 1] ** 2) / (2.0 * ell2)
    T0h, T0m, T0l = _split3(T0)
    u0 = xs[:, 0] / ell2
    u0h, u0l = _split2(u0)
    u1 = xs[:, 1] / ell2
    u1h, u1l = _split2(u1)
    onesM = np.ones(M, dtype=ml_dtypes.bfloat16)
    B = np.stack(
        [T0h, T0m, T0l, u0h, u0l, u0h, u0l, u1h, u1l, u1h, u1l, onesM, onesM, onesM]
    )

    # alpha tiles (TT, nr*4): hi/lo split of each alpha column, tile-major
    arh, arl = _split2(al[:, 0])
    aih, ail = _split2(al[:, 1])
    AL = np.stack([arh, arl, aih, ail], axis=1)  # (N, 4)
    AL = AL.reshape(N // TT, TT, 4)

    # Gather per-core inputs from the schedule: per-slot [B | A | AL]
    W = TC + T * TT + T * 4
    in_maps, placements = [], []
    for c in range(NCORES):
        IN_g = np.zeros((CP, S * W), dtype=ml_dtypes.bfloat16)
        place = []
        for s in range(S):
            leaf, tiles = entries[s * NCORES + c]
            bleaf = leaf if leaf >= 0 else 0
            col = s * W
            IN_g[:C, col:col + TC] = B[:, bleaf * TC:(bleaf + 1) * TC]
            place.append(leaf)
            for t in range(T):
                tile = int(tiles[t]) if t < len(tiles) else 0
                IN_g[:C, col + TC + t * TT:col + TC + (t + 1) * TT] = (
                    A[:, tile * TT:(tile + 1) * TT]
                )
                if t < len(tiles):
                    IN_g[:TT, col + TC + T * TT + t * 4:
                         col + TC + T * TT + (t + 1) * 4] = AL[tile].reshape(TT, 4)
        in_maps.append({"inp": IN_g})
        placements.append(place)
    return in_maps, placements, perm_t, S, T, bias


def _combine(results, placements, perm_t, S):
    mu_sorted = np.zeros((M, 2), dtype=np.float32)
    for c in range(NCORES):
        o = results[c]["out"]
        for s, leaf in enumerate(placements[c]):
            if leaf < 0:
                continue
            sl = slice(leaf * TC, (leaf + 1) * TC)
            mu_sorted[sl, 0] += o[0, s * TC:(s + 1) * TC] + o[1, s * TC:(s + 1) * TC]
            mu_sorted[sl, 1] += o[2, s * TC:(s + 1) * TC] + o[3, s * TC:(s + 1) * TC]
    out = np.empty((M, 2), dtype=np.float32)
    out[perm_t] = mu_sorted
    return out


def kernel(X_test, X_train, alpha, log_lengthscale, log_outputscale):
    from concourse.bass_utils import run_bass_kernel_spmd

    in_maps, placements, perm_t, S, T, bias = _prep(
        X_test, X_train, alpha, log_lengthscale, log_outputscale
    )
    key = (S, T, float(bias))
    if key not in _cache:
        _cache[key] = _build_program(bias, S, T)
    nc = _cache[key]

    core_ids = list(range(NCORES))
    res = run_bass_kernel_spmd(nc, in_maps, core_ids)
    return _combine(res.results, placements, perm_t, S)
